# revision 17
# baseline (speedup 1.0000x reference)
"""Trainium2 Bass kernel: batched Ising energies E_b = s_b^T J s_b.

state: [1024, 2048] float32 in {0,1};  J: [2048, 2048] float32.
Returns energies [1024] float32.

Strategy (8 NeuronCores): sharding is 2D, 4 column-blocks of J x 2
batch-halves.  Core (r, c) computes, for its batch half and J block,
partial_rc[b] = sum_{j in cols_r} (spins[b,:] @ J[:, j]) * spins[b, j]
via PE matmuls (contraction over all 2048 rows of J) plus a
multiply+reduce on the vector engine.  The host sums the 4 column-block
partials per batch half - no on-device collectives.

J precision modes:
  "f32r": J streamed as FP32R (fp32 truncated to ~FP22 inside the PE,
          full matmul rate for moving dim >= 256).  state ships as
          uint8 and is expanded to +-1 fp32 spins on the otherwise-idle
          vector engine (PE matmul requires both operands 32-bit).
  "hilo": J = bf16 hi + bf16 lo, two accumulated matmul passes
          (fp32-level accuracy, 2x matmul work)
  "hi":   J as bf16 only (fastest, bf16-level accuracy)

All device inputs are pre-arranged on the host into [128, X] layouts that
are contiguous per SBUF partition, so DMA descriptors are 4-16KB and the
loads run near HBM rate.  Loads are split across the sync and scalar
HWDGE rings.  A burst of small dummy matmuls on a zeroed tile warms the
PE clock gate (HAM) while the loads are in flight.
"""

import sys

if "/opt/trn_rl_repo" not in sys.path:
    sys.path.insert(0, "/opt/trn_rl_repo")

import numpy as np
import ml_dtypes

B, N = 1024, 2048
R, C = 4, 2          # J column-block split x batch split (R*C = 8 cores)
CB = N // R          # 512 J-columns per core
BH = B // C          # 512 samples per core
P = 128
KT = N // P          # 16 contraction tiles
BT = BH // P         # 4 output-partition tiles
CHUNK = 4            # k-tiles per input DMA
N_WARM = 120         # small dummy matmuls to warm the PE clock gate
WARM_N = 64          # free dim of each warmup matmul
K_TAIL = 4           # k-tiles computed per-b at the end (epilogue stagger)
MODE = "f32r"        # "f32r" | "hilo" | "hi"

_cache = {}


def _build_program():
    import concourse.bacc as bacc
    import concourse.mybir as mybir
    import concourse.tile as tile

    bf16 = mybir.dt.bfloat16
    f32 = mybir.dt.float32
    f32r = mybir.dt.float32r
    u8 = mybir.dt.uint8
    jdt = f32r if MODE == "f32r" else bf16

    nc = bacc.Bacc("TRN2", target_bir_lowering=False, debug=False, num_devices=R * C)

    if MODE == "f32r":
        su_ext = nc.dram_tensor("su", [P, KT * BH], u8, kind="ExternalInput").ap()
    else:
        st_ext = nc.dram_tensor("st", [P, KT * BH], bf16, kind="ExternalInput").ap()
    jhi_ext = nc.dram_tensor("jhi", [P, KT * CB], jdt, kind="ExternalInput").ap()
    jlo_ext = (
        nc.dram_tensor("jlo", [P, KT * CB], bf16, kind="ExternalInput").ap()
        if MODE == "hilo"
        else None
    )
    sb_ext = nc.dram_tensor("sb", [P, BT * CB], bf16, kind="ExternalInput").ap()
    out_ext = nc.dram_tensor("part", [BH], f32, kind="ExternalOutput").ap()

    with tile.TileContext(nc) as tc:
        with (
            tc.tile_pool(name="persist", bufs=1) as persist,
            tc.tile_pool(name="work", bufs=3) as work,
            tc.tile_pool(name="psum", bufs=1, space="PSUM") as psum_pool,
            tc.tile_pool(name="warmps", bufs=1, space="PSUM") as warm_pool,
        ):
            sdt = f32r if MODE == "f32r" else bf16
            st_t = persist.tile([P, KT, BH], sdt)
            su_t = (
                persist.tile([P, KT, BH], u8, name="su_t")
                if MODE == "f32r"
                else None
            )
            jhi_t = persist.tile([P, KT, CB], jdt)
            jlo_t = (
                persist.tile([P, KT, CB], bf16, name="jlo_t")
                if MODE == "hilo"
                else None
            )
            sb_t = persist.tile([P, BT, CB], bf16)
            red_all = persist.tile([P, BT], f32)
            warm_src = persist.tile([P, CB], bf16)

            # PE warmup: small dummy matmuls on a zeroed tile keep the HAM
            # activity window busy while the real loads stream in.  Small
            # free dim => the last one never delays the first real matmul.
            nc.vector.memset(warm_src[:], 0.0)
            warm_ps = warm_pool.tile([P, WARM_N], f32)
            for _ in range(N_WARM):
                nc.tensor.matmul(
                    warm_ps, lhsT=warm_src[:, :P], rhs=warm_src[:, :WARM_N],
                    start=True, stop=True,
                )

            # Input loads: chunks with 4-16KB per-partition contiguous
            # runs, split across the sync and scalar HWDGE rings.  The
            # first-needed data (state halves + J chunk 0) goes on the
            # sync ring, which has the shorter first-byte latency.
            n_chunks = KT // CHUNK
            if MODE == "f32r":
                half = KT * BH // 2
                nc.sync.dma_start(
                    out=su_t[:, : KT // 2], in_=su_ext[:, :half]
                )
                nc.sync.dma_start(
                    out=su_t[:, KT // 2 :], in_=su_ext[:, half:]
                )
                for ci in range(n_chunks):
                    kt = slice(ci * CHUNK, (ci + 1) * CHUNK)
                    kc = slice(ci * CHUNK * CB, (ci + 1) * CHUNK * CB)
                    eng = nc.sync if ci % 2 == 0 else nc.scalar
                    eng.dma_start(out=jhi_t[:, kt], in_=jhi_ext[:, kc])
                # expand uint8 {0,1} -> +-1.0 spins on the vector engine
                for ci in range(n_chunks):
                    kt = slice(ci * CHUNK, (ci + 1) * CHUNK)
                    nc.vector.tensor_scalar(
                        st_t[:, kt],
                        su_t[:, kt],
                        2.0,
                        -1.0,
                        mybir.AluOpType.mult,
                        mybir.AluOpType.add,
                    )
            else:
                for ci in range(n_chunks):
                    kt = slice(ci * CHUNK, (ci + 1) * CHUNK)
                    ks = slice(ci * CHUNK * BH, (ci + 1) * CHUNK * BH)
                    kc = slice(ci * CHUNK * CB, (ci + 1) * CHUNK * CB)
                    nc.sync.dma_start(out=st_t[:, kt], in_=st_ext[:, ks])
                    nc.scalar.dma_start(out=jhi_t[:, kt], in_=jhi_ext[:, kc])
            nc.scalar.dma_start(out=sb_t[:], in_=sb_ext.rearrange(
                "p (t c) -> p t c", c=CB))
            if MODE == "hilo":
                for ci in range(n_chunks):
                    kt = slice(ci * CHUNK, (ci + 1) * CHUNK)
                    kc = slice(ci * CHUNK * CB, (ci + 1) * CHUNK * CB)
                    eng = nc.sync if ci % 2 == 0 else nc.scalar
                    eng.dma_start(out=jlo_t[:, kt], in_=jlo_ext[:, kc])

            ps_tiles = [
                psum_pool.tile([P, CB], f32, name=f"ps_{b}") for b in range(BT)
            ]

            def mm(b, k, jt, start, stop):
                nc.tensor.matmul(
                    ps_tiles[b],
                    lhsT=st_t[:, k, b * P : (b + 1) * P],
                    rhs=jt[:, k],
                    start=start,
                    stop=stop,
                )

            def epilogue(b):
                m = work.tile([P, CB], f32, name="m_epi")
                nc.vector.tensor_tensor(
                    m[:], ps_tiles[b][:], sb_t[:, b], mybir.AluOpType.mult
                )
                nc.vector.tensor_reduce(
                    red_all[:, b : b + 1],
                    m[:],
                    mybir.AxisListType.X,
                    mybir.AluOpType.add,
                )

            if MODE == "hilo":
                # hi pass k-outer (4 matmuls runnable per arriving chunk),
                # then lo pass b-outer so epilogues overlap remaining MMs
                for k in range(KT):
                    for b in range(BT):
                        mm(b, k, jhi_t, start=(k == 0), stop=False)
                for b in range(BT):
                    for k in range(KT):
                        mm(b, k, jlo_t, start=False, stop=(k == KT - 1))
                    epilogue(b)
            else:
                # single pass: k-outer for the bulk, the last K_TAIL
                # k-tiles go b-by-b so epilogues overlap the tail matmuls
                for k in range(KT - K_TAIL):
                    for b in range(BT):
                        mm(b, k, jhi_t, start=(k == 0), stop=False)
                for b in range(BT):
                    for k in range(KT - K_TAIL, KT):
                        mm(b, k, jhi_t, start=False, stop=(k == KT - 1))
                    epilogue(b)

            nc.sync.dma_start(
                out=out_ext.rearrange("(t p) -> p t", p=P), in_=red_all[:]
            )

    nc.compile()
    return nc


def _part_layout(a, inner):
    """[KT*P, inner] row-major -> [P, KT*inner] contiguous per partition."""
    k = a.shape[0] // P
    return np.ascontiguousarray(
        a.reshape(k, P, inner).transpose(1, 0, 2).reshape(P, k * inner)
    )


def _make_in_maps(state, J):
    bf16 = ml_dtypes.bfloat16
    state = np.asarray(state, dtype=np.float32)
    J = np.asarray(J, dtype=np.float32)

    spins = state * 2.0 - 1.0                       # exact in fp32
    sp_bf = spins.astype(bf16)                      # [B, N], exact (+-1)
    if MODE == "f32r":
        su_all = state.astype(np.uint8).T           # [N, B] {0,1}
        Jhi = J
    else:
        st_all = sp_bf.T                            # [N, B] view
        Jhi = J.astype(bf16)
        if MODE == "hilo":
            Jlo = (J - Jhi.astype(np.float32)).astype(bf16)

    in_maps = []
    placement = []
    for core in range(R * C):
        r, c = divmod(core, C)
        m = {
            "jhi": _part_layout(Jhi[:, r * CB : (r + 1) * CB], CB),
            "sb": _part_layout(
                sp_bf[c * BH : (c + 1) * BH, r * CB : (r + 1) * CB], CB
            ),
        }
        if MODE == "f32r":
            m["su"] = _part_layout(su_all[:, c * BH : (c + 1) * BH], BH)
        else:
            m["st"] = _part_layout(st_all[:, c * BH : (c + 1) * BH], BH)
        if MODE == "hilo":
            m["jlo"] = _part_layout(Jlo[:, r * CB : (r + 1) * CB], CB)
        in_maps.append(m)
        placement.append((r, c))
    return in_maps, placement


def kernel(state, J):
    from concourse.bass_utils import run_bass_kernel_spmd

    if "nc" not in _cache:
        _cache["nc"] = _build_program()
    nc = _cache["nc"]

    in_maps, placement = _make_in_maps(state, J)
    res = run_bass_kernel_spmd(nc, in_maps, list(range(R * C)))

    out = np.zeros(B, dtype=np.float32)
    for core, (r, c) in enumerate(placement):
        out[c * BH : (c + 1) * BH] += res.results[core]["part"]
    return out
